# revision 35
# baseline (speedup 1.0000x reference)
"""GCN layer (nn_GCNReg) on 8 Trainium2 NeuronCores.

Strategy (graph/data parallel, per sharding hint):
  - Nodes are partitioned across 8 cores by destination range; edges are
    routed to the core owning their dst and sorted by (dst batch, src).
  - Math: out = relu(dinv_dst * ((sum_e xs[src_e] * S) @ W1^T) + b1) @ W2^T
    + b2 with xs = dinv[:,None] * x computed on host (f16 gather table,
    replicated to every core's HBM).  The W1 transform commutes with the
    aggregation, so each core only transforms its own 6272 aggregated rows.
  - Per 512-dst batch the core gathers its edges' source rows with
    dma_gather (round-robin across 4 SWDGE queues so descriptor generation
    runs on all 8 Q7 cores), and accumulates agg^T = sum_chunk M_chunk^T
    S_chunk in PSUM via TensorE.  S matrices are host-built one-hot
    scatter matrices streamed from HBM; self-loop chunks use an identity S
    against a resident tile of the core's own xs rows, so they need no
    gather at all.
  - dinv_dst is applied post-transform via a host-shipped broadcast table.
"""

import sys

import numpy as np

for _p in ("/opt/trn_rl_repo", "/opt/pypackages"):
    if _p not in sys.path:
        sys.path.append(_p)

import concourse.bass as bass
import concourse.tile as tile
from concourse import bacc, mybir
from concourse.bass_utils import run_bass_kernel_spmd

N = 50000
D = 128
HID = 128
ODIM = 8
CORES = 8
TILE = 128
TPC = 49                      # dst tiles per core
NPC = TPC * TILE              # 6272 nodes per core
NP = CORES * NPC              # 50176 padded node count
LO = 32768                    # int16-safe gather-table split
BATCH_BWS = [512] * 11 + [256, 256, 128]   # 6272; small batches last
BATCH_OFF = [sum(BATCH_BWS[:i]) for i in range(len(BATCH_BWS))]
NBATCH = len(BATCH_BWS)
GCHUNKS = 8                   # chunks per dma_gather call (1024 idxs)
NQ = 4                        # SWDGE queues (Q7 core pairs)

F16 = np.float16
F32 = np.float32


def _preprocess(edge_index):
    """Route/sort edges; build uniform compile-time meta + per-core arrays."""
    src = np.asarray(edge_index[0], dtype=np.int64).astype(np.int32)
    dst = np.asarray(edge_index[1], dtype=np.int64).astype(np.int32)

    deg = np.bincount(dst, minlength=NP).astype(np.int64) + 1  # + self-loop
    dinv = (1.0 / np.sqrt(deg.astype(np.float64))).astype(F32)

    order = np.argsort(dst, kind="stable")
    ssrc = src[order]
    sdst = dst[order]
    counts = np.bincount(sdst, minlength=NP)
    ptr = np.zeros(NP + 1, dtype=np.int64)
    ptr[1:] = np.cumsum(counts)

    # per (core, batch, side): src-index list + batch-relative dst list
    per = [[None] * (2 * NBATCH) for _ in range(CORES)]
    for c in range(CORES):
        for b in range(NBATCH):
            base = c * NPC + BATCH_OFF[b]
            bw = BATCH_BWS[b]
            lo_e = ptr[base]
            hi_e = ptr[base + bw]
            s = ssrc[lo_e:hi_e]
            dl = (sdst[lo_e:hi_e] - base).astype(np.int32)
            m = s < LO
            per[c][2 * b] = (s[m], dl[m])
            per[c][2 * b + 1] = (s[~m] - LO, dl[~m])

    # uniform chunk counts + mm-entry lists
    meta = {"batches": []}
    nidx16_tot = 0
    nmm_tot = 0         # host-shipped S columns (gathered-chunk entries only)
    for b in range(NBATCH):
        bw = BATCH_BWS[b]
        ent = {"bw": bw, "sides": [], "col0": nmm_tot}
        for side in range(2):
            cmax = max(len(per[c][2 * b + side][0]) for c in range(CORES))
            cmax = max(cmax, 1)
            k = (cmax + TILE - 1) // TILE
            ent["sides"].append(
                {"k": k, "cmax": cmax, "idx_off16": nidx16_tot}
            )
            nidx16_tot += (k * TILE) // 16

        # union dst-tile span per chunk across cores -> per-tile mm entries
        mm = []  # gathered entries: (buf_ci, tile_j, col)
        klo = ent["sides"][0]["k"]
        for side in range(2):
            k = ent["sides"][side]["k"]
            lo_span = np.full(k, np.inf)
            hi_span = np.full(k, -np.inf)
            for c in range(CORES):
                _, dl_l = per[c][2 * b + side]
                n = len(dl_l)
                if n == 0:
                    continue
                nk = (n + TILE - 1) // TILE
                starts = np.arange(nk) * TILE
                mn = np.minimum.reduceat(dl_l, starts)
                mx = np.maximum.reduceat(dl_l, starts)
                lo_span[:nk] = np.minimum(lo_span[:nk], mn)
                hi_span[:nk] = np.maximum(hi_span[:nk], mx)
            for ci in range(k):
                if not np.isfinite(lo_span[ci]):
                    continue   # all-pad chunk on every core: no matmul
                j0 = int(lo_span[ci]) // TILE
                j1 = int(hi_span[ci]) // TILE
                buf_ci = ci if side == 0 else klo + ci
                for j in range(j0, j1 + 1):
                    mm.append((buf_ci, j, nmm_tot))
                    nmm_tot += 1
        ent["mm"] = mm
        meta["batches"].append(ent)
    meta["nidx16"] = nidx16_tot
    meta["nmm"] = max(nmm_tot, 1)
    meta["maxch"] = max(
        e["sides"][0]["k"] + e["sides"][1]["k"] for e in meta["batches"]
    )
    meta["maxmm"] = max(max(len(e["mm"]) for e in meta["batches"]), 1)

    # per-core packed arrays
    import ml_dtypes
    smat = np.zeros((CORES, 128, meta["nmm"], TILE),
                    dtype=ml_dtypes.float8_e4m3fn)
    srcidx = np.zeros((CORES, 128, nidx16_tot), dtype=np.int16)
    for c in range(CORES):
        for b in range(NBATCH):
            ent = meta["batches"][b]
            # col lookup per (buf_ci, j)
            kall = ent["sides"][0]["k"] + ent["sides"][1]["k"]
            colmap = np.full((kall, TPC), -1, dtype=np.int64)
            for buf_ci, j, col in ent["mm"]:
                colmap[buf_ci, j] = col
            for side in range(2):
                sd = ent["sides"][side]
                idx_l, dl_l = per[c][2 * b + side]
                n = len(idx_l)
                k = sd["k"]
                tot = k * TILE
                idx = np.zeros(tot, dtype=np.int16)
                idx[:n] = idx_l.astype(np.int16)
                srcidx[c][:, sd["idx_off16"] : sd["idx_off16"] + tot // 16] = (
                    np.tile(idx.reshape(tot // 16, 16).T, (8, 1))
                )
                if n == 0:
                    continue
                slots = np.arange(n)
                ci = slots // TILE + (0 if side == 0 else ent["sides"][0]["k"])
                m = slots % TILE
                j = dl_l // TILE
                cols = colmap[ci, j]
                assert (cols >= 0).all()
                smat[c][m, cols, dl_l % TILE] = 1.0

    return meta, dinv, srcidx, smat


def _build_program(meta):
    nc = bacc.Bacc("TRN2", target_bir_lowering=False, debug=False,
                   num_devices=CORES, num_swdge_queues=NQ)
    dt = mybir.dt

    xs_d = nc.dram_tensor("xs", [NP, D], dt.float16, kind="ExternalInput")
    selfx_d = nc.dram_tensor("selfx", [128, TPC * D], dt.float16,
                             kind="ExternalInput")
    smat_d = nc.dram_tensor("smat", [128, meta["nmm"] * TILE], dt.float8e4,
                            kind="ExternalInput")
    srcidx_d = nc.dram_tensor("srcidx", [128, meta["nidx16"]], dt.int16,
                              kind="ExternalInput")
    ident_d = nc.dram_tensor("ident", [128, 128], dt.float16,
                             kind="ExternalInput")
    dinvB_d = nc.dram_tensor("dinvB", [128, NPC], dt.float16,
                             kind="ExternalInput")
    w1t_d = nc.dram_tensor("w1t", [D, HID], dt.float32, kind="ExternalInput")
    b1_d = nc.dram_tensor("b1c", [HID, 1], dt.float32, kind="ExternalInput")
    w2t_d = nc.dram_tensor("w2t", [HID, ODIM], dt.float16, kind="ExternalInput")
    b2_d = nc.dram_tensor("b2c", [ODIM, 1], dt.float32, kind="ExternalInput")
    out_d = nc.dram_tensor("out", [ODIM, NPC], dt.float32, kind="ExternalOutput")

    with tile.TileContext(nc) as tc:
        with (
            tc.tile_pool(name="const", bufs=1) as cpool,
            tc.tile_pool(name="smat", bufs=3) as s_pool,
            tc.tile_pool(name="msg", bufs=4) as msg_pool,
            tc.tile_pool(name="eptmp", bufs=2) as ep_pool,
            tc.tile_pool(name="psA", bufs=2, space="PSUM") as psA,
            tc.tile_pool(name="psZ", bufs=2, space="PSUM") as psZ,
            tc.tile_pool(name="psO", bufs=2, space="PSUM") as psO,
        ):
            # ---- srcidx first: the only dependency of the first gathers.
            # Split so batch 0's gathers wait only on its own slice.
            n0 = meta["batches"][1]["sides"][0]["idx_off16"]
            idx_a = cpool.tile([128, n0], dt.int16, tag="srcidxa")
            nc.sync.dma_start(idx_a[:], srcidx_d.ap()[:, :n0])
            idx_b = cpool.tile([128, meta["nidx16"] - n0], dt.int16,
                               tag="srcidxb")
            nc.sync.dma_start(idx_b[:], srcidx_d.ap()[:, n0:])

            def idx_ap(off16, len16):
                if off16 < n0:
                    return idx_a[:, off16 : off16 + len16]
                return idx_b[:, off16 - n0 : off16 - n0 + len16]

            xs_lo_ap = xs_d.ap()[0:LO, :]
            xs_hi_ap = xs_d.ap()[LO:NP, :]
            qload = [0] * NQ  # per-queue gather idx totals (least-loaded pick)

            def issue_gathers(ent, buf):
                klo = ent["sides"][0]["k"]
                for side, c0 in ((0, 0), (1, klo)):
                    sd = ent["sides"][side]
                    k = sd["k"]
                    for p0 in range(0, k, GCHUNKS):
                        pk = min(GCHUNKS, k - p0)
                        off = sd["idx_off16"] + (p0 * TILE) // 16
                        # always gather full chunks: pad slots use idx 0, so
                        # every slot a scatter matmul can read holds finite
                        # data (a trimmed call would leave stale SBUF in its
                        # tail, and 0 * NaN = NaN whenever the same pool
                        # buffer previously covered fewer chunks)
                        ni = pk * TILE
                        q = qload.index(min(qload))
                        qload[q] += ni
                        nc.gpsimd.dma_gather(
                            out_ap=buf[:, c0 + p0 : c0 + p0 + pk, :],
                            in_ap=xs_lo_ap if side == 0 else xs_hi_ap,
                            idxs_ap=idx_ap(off, ni // 16),
                            num_idxs=ni,
                            num_idxs_reg=ni,
                            elem_size=D,
                            single_packet=True,
                            queue_num=q,
                        )

            # batch 0 gathers go ahead of every other transfer
            bufs = {}
            bufs[0] = msg_pool.tile([128, meta["maxch"], D], dt.float16,
                                    tag="msg", name="msgbuf")
            issue_gathers(meta["batches"][0], bufs[0])

            # ---- remaining constants (consumed later than the gathers) ----
            ident_t = cpool.tile([128, 128], dt.float16, tag="ident")
            nc.sync.dma_start(ident_t[:], ident_d.ap())
            selfx_t = cpool.tile([128, TPC, D], dt.float16, tag="selfx")
            nc.sync.dma_start(
                selfx_t[:], selfx_d.ap().rearrange("p (a d) -> p a d", d=D)
            )
            dinvB_t = cpool.tile([128, NPC], dt.float16, tag="dinvB")
            nc.sync.dma_start(dinvB_t[:], dinvB_d.ap())
            w1t_t = cpool.tile([D, HID], dt.float32, tag="w1t")
            nc.sync.dma_start(w1t_t[:], w1t_d.ap())
            b1_t = cpool.tile([HID, 1], dt.float32, tag="b1")
            nc.sync.dma_start(b1_t[:], b1_d.ap())
            w2t_t = cpool.tile([HID, ODIM], dt.float16, tag="w2t")
            nc.sync.dma_start(w2t_t[:], w2t_d.ap())
            b2_t = cpool.tile([ODIM, 1], dt.float32, tag="b2")
            nc.sync.dma_start(b2_t[:], b2_d.ap())
            zeros_t = cpool.tile([1, 512], dt.float16, tag="zeros")
            nc.vector.memset(zeros_t[:], 0.0)

            for b in range(NBATCH):
                ent = meta["batches"][b]
                bw = ent["bw"]
                mm = ent["mm"]
                nmm_b = len(mm)
                col0 = ent["col0"]

                if b not in bufs:
                    bufs[b] = msg_pool.tile([128, meta["maxch"], D],
                                            dt.float16, tag="msg",
                                            name="msgbuf")
                    issue_gathers(ent, bufs[b])
                buf = bufs[b]
                if b + 1 < NBATCH:
                    bufs[b + 1] = msg_pool.tile([128, meta["maxch"], D],
                                                dt.float16, tag="msg",
                                                name="msgbuf")
                    issue_gathers(meta["batches"][b + 1], bufs[b + 1])

                s_t = s_pool.tile([128, meta["maxmm"], TILE], dt.float8e4,
                                  tag="smat")
                if nmm_b:
                    nc.sync.dma_start(
                        s_t[:, :nmm_b, :],
                        smat_d.ap()[:, col0 * TILE : (col0 + nmm_b) * TILE]
                        .rearrange("p (m d) -> p m d", d=TILE),
                    )

                # aggregate: bank-wide zero init, then self-loop identity
                # matmuls and gathered chunks accumulate (order-free)
                agg_ps = psA.tile([128, 512], dt.float32, tag="agg")
                nc.tensor.matmul(
                    out=agg_ps[:], lhsT=zeros_t[:, :128], rhs=zeros_t[:],
                    start=True, stop=False, skip_group_check=True,
                )
                ntile = bw // TILE
                g0 = BATCH_OFF[b] // TILE
                for a in range(ntile):
                    nc.tensor.matmul(
                        out=agg_ps[:, a * TILE : (a + 1) * TILE],
                        lhsT=selfx_t[:, g0 + a, :],
                        rhs=ident_t[:],
                        start=False,
                        stop=(nmm_b == 0 and a == ntile - 1),
                        skip_group_check=True,
                    )
                for i, (buf_ci, j, _) in enumerate(mm):
                    nc.tensor.matmul(
                        out=agg_ps[:, j * TILE : (j + 1) * TILE],
                        lhsT=buf[:, buf_ci, :],
                        rhs=s_t[:, i, :],
                        start=False,
                        stop=(i == nmm_b - 1),
                        skip_group_check=True,
                    )

                # epilogue for this batch
                agg_sb = ep_pool.tile([128, 512], dt.float32, tag="aggsb")
                nc.scalar.copy(agg_sb[:, :bw], agg_ps[:, :bw])
                z_ps = psZ.tile([128, 512], dt.float32, tag="z")
                nc.tensor.matmul(out=z_ps[:, :bw], lhsT=w1t_t[:],
                                 rhs=agg_sb[:, :bw], start=True, stop=True)
                z2_sb = ep_pool.tile([128, 512], dt.float32, tag="z2")
                nc.vector.tensor_tensor(
                    out=z2_sb[:, :bw],
                    in0=z_ps[:, :bw],
                    in1=dinvB_t[:, BATCH_OFF[b] : BATCH_OFF[b] + bw],
                    op=mybir.AluOpType.mult,
                )
                h_sb = ep_pool.tile([128, 512], dt.float16, tag="h")
                nc.scalar.activation(h_sb[:, :bw], z2_sb[:, :bw],
                                     mybir.ActivationFunctionType.Relu,
                                     bias=b1_t[:])
                o_ps = psO.tile([ODIM, 512], dt.float32, tag="o")
                nc.tensor.matmul(out=o_ps[:, :bw], lhsT=w2t_t[:],
                                 rhs=h_sb[:, :bw], start=True, stop=True)
                o_sb = ep_pool.tile([ODIM, 512], dt.float32, tag="osb")
                nc.vector.tensor_scalar(
                    out=o_sb[:, :bw],
                    in0=o_ps[:, :bw],
                    scalar1=b2_t[:],
                    scalar2=None,
                    op0=mybir.AluOpType.add,
                )
                nc.sync.dma_start(
                    out_d.ap()[:, BATCH_OFF[b] : BATCH_OFF[b] + bw],
                    o_sb[:, :bw]
                )

    nc.compile()
    return nc


_CACHE = {}
last_results = None


def kernel(x, edge_index, W1, b1, W2, b2):
    import os

    meta, dinv, srcidx, smat = _preprocess(edge_index)

    # host-prescaled f16 gather table: xs = dinv * x (padded to NP rows)
    xs = np.zeros((NP, D), dtype=F16)
    xs[:N] = (np.asarray(x, dtype=F32) * dinv[:N, None]).astype(F16)
    ident = np.eye(128, dtype=F16)
    w1t = np.asarray(W1, dtype=F32).T.copy()              # [D, HID]
    b1c = np.asarray(b1, dtype=F32).reshape(HID, 1)
    w2t = np.asarray(W2, dtype=F32).T.astype(F16).copy()  # [HID, ODIM]
    b2c = np.asarray(b2, dtype=F32).reshape(ODIM, 1)

    key = tuple(
        (e["bw"], tuple(e["mm"]))
        + tuple((sd["k"], sd["idx_off16"]) for sd in e["sides"])
        for e in meta["batches"]
    )
    if key not in _CACHE:
        _CACHE[key] = _build_program(meta)
    nc = _CACHE[key]

    in_maps = []
    for c in range(CORES):
        rows = xs[c * NPC : (c + 1) * NPC]               # [NPC, D]
        selfx = np.ascontiguousarray(
            rows.reshape(TPC, 128, D).transpose(1, 0, 2).reshape(128, TPC * D)
        )
        dinvB = np.broadcast_to(
            dinv[c * NPC : (c + 1) * NPC].astype(F16), (128, NPC)
        ).copy()
        in_maps.append(
            {
                "xs": xs,
                "selfx": selfx,
                "smat": smat[c].reshape(128, meta["nmm"] * TILE),
                "srcidx": srcidx[c],
                "ident": ident,
                "dinvB": dinvB,
                "w1t": w1t,
                "b1c": b1c,
                "w2t": w2t,
                "b2c": b2c,
            }
        )

    trace = bool(os.environ.get("GCN_TRACE"))
    res = run_bass_kernel_spmd(
        nc, in_maps, core_ids=list(range(CORES)), trace=trace
    )
    global last_results
    last_results = res
    big = np.concatenate([res.results[c]["out"] for c in range(CORES)], axis=1)
    return np.ascontiguousarray(big[:, :N].T).astype(F32)
